# revision 1
# baseline (speedup 1.0000x reference)
"""Trainium2 Bass kernel: 16-head RoPE attention block (B=4, T=2048, D=2048).

Sharding: tensor-parallel over heads. Each of the 8 cores owns 2 heads
(a 256-wide slice of the q/k/v projection output features). Per core:

  stage 1: q/k/v projections in feature-major layout (stationary = W^T
           tiles, moving = x^T), RoPE applied to q/k on the vector engine,
           v transposed to token-major via the PE; results staged in DRAM
           (per-batch scratch tiles so stage 2 can start per batch).
  stage 2: per (batch, head): scores computed TRANSPOSED (S^T[k,q] =
           kTile^T @ qT) so softmax->PV needs no P transpose; exp on the
           scalar engine (no max subtraction needed: scores ~ N(0,1));
           PV + a ones-row matmul (softmax denominators) accumulate on
           the PE interleaved with the score matmuls; normalization via
           reciprocal broadcast.
  stage 3: out-projection partial product (full D columns) feature-major.

Host sums the 8 partial outputs (the "all-reduce") and un-transposes.
All matmuls run in float32r (FP22 multiply, fp32 accumulate): full PE
throughput with ~1e-4 relative error.
"""

import math

import numpy as np

import concourse.bacc as bacc
import concourse.bass as bass
import concourse.mybir as mybir
import concourse.tile as tile
from concourse.bass_utils import run_bass_kernel_spmd

F32 = mybir.dt.float32
F32R = mybir.dt.float32r
EXP = mybir.ActivationFunctionType.Exp

# Problem shape (hardcoded; the harness calls kernel() with exactly these).
B = 4
T = 2048
D_MODEL = 2048
HEAD_DIM = 128
N_CORES = 8
ROPE_BASE = 10000.0

HPC = 2                      # heads per core
F_LOC = HPC * HEAD_DIM       # 256 local projection features per core
BT = B * T
TCH = 512                    # token chunk width (stages 1/3)
QCH = 512                    # query chunk width (stage 2)
SCALE = 1.0 / math.sqrt(HEAD_DIM)
S_LOOK = 3                   # score-matmul lookahead in the attention loop


def build_module(b=B, t=T, d_model=D_MODEL, n_cores=N_CORES):
    """Build the per-core Bass module. All cores run the same program on
    different data (pure SPMD, no collectives)."""
    bt = b * t
    dt_ = d_model // 128
    kt = t // 128
    tch = min(TCH, bt)
    qch = min(QCH, t)
    ntch = bt // tch
    nqc = t // qch
    cpb = t // tch           # stage-1/3 token chunks per batch

    nc = bacc.Bacc(None, target_bir_lowering=False)

    xT = nc.dram_tensor("xT", [d_model, bt], F32, kind="ExternalInput")
    wqT = nc.dram_tensor("wqT", [d_model, F_LOC], F32, kind="ExternalInput")
    wkT = nc.dram_tensor("wkT", [d_model, F_LOC], F32, kind="ExternalInput")
    wvT = nc.dram_tensor("wvT", [d_model, F_LOC], F32, kind="ExternalInput")
    woT = nc.dram_tensor("woT", [F_LOC, d_model], F32, kind="ExternalInput")
    cosT = nc.dram_tensor("cosT", [HEAD_DIM, t], F32, kind="ExternalInput")
    rsinT = nc.dram_tensor("rsinT", [HEAD_DIM, t], F32, kind="ExternalInput")
    ident = nc.dram_tensor("ident", [128, 128], F32, kind="ExternalInput")
    onesc = nc.dram_tensor("onesc", [128, 1], F32, kind="ExternalInput")
    outP = nc.dram_tensor("outP", [d_model, bt], F32, kind="ExternalOutput")

    with tile.TileContext(nc) as tc:
        with (
            tc.tile_pool(name="const", bufs=1) as constp,
            tc.tile_pool(name="dram", bufs=1, space="DRAM") as dram,
            tc.tile_pool(name="ps_mm", bufs=4, space="PSUM") as ps_mm,
            tc.tile_pool(name="ps_pv", bufs=2, space="PSUM") as ps_pv,
            tc.tile_pool(name="ps_dn", bufs=2, space="PSUM") as ps_dn,
        ):
            # ---- constants (gpsimd ring: keep the sync ring free for the
            # stage-1 weight/x loads that gate the first matmuls) ----
            cos_sb = constp.tile([128, t], F32)
            nc.gpsimd.dma_start(out=cos_sb, in_=cosT[:, :])
            rsin_sb = constp.tile([128, t], F32)
            nc.gpsimd.dma_start(out=rsin_sb, in_=rsinT[:, :])
            id_sb = constp.tile([128, 128], F32)
            nc.gpsimd.dma_start(out=id_sb, in_=ident[:, :])
            ones_sb = constp.tile([128, 1], F32R)
            nc.gpsimd.dma_start(out=ones_sb, in_=onesc[:, :].bitcast(F32R))

            # ---- DRAM scratch (per (head, batch) so cross-stage deps are
            # batch-granular and the stages can pipeline) ----
            q_scr = [
                [dram.tile([128, t], F32, name=f"qs{h}_{bi}", tag=f"qs{h}_{bi}") for bi in range(b)]
                for h in range(HPC)
            ]
            k_scr = [
                [dram.tile([128, t], F32, name=f"ks{h}_{bi}", tag=f"ks{h}_{bi}") for bi in range(b)]
                for h in range(HPC)
            ]
            v_scr = [
                [dram.tile([kt, 128, 128], F32, name=f"vs{h}_{bi}", tag=f"vs{h}_{bi}") for bi in range(b)]
                for h in range(HPC)
            ]
            den_dram = dram.tile([b * HPC, t], F32)
            rec_dram = dram.tile([b * HPC, t], F32)

            # ================= stage 1: projections + rope + v^T =========
            with (
                tc.tile_pool(name="s1w", bufs=1) as wpool,
                tc.tile_pool(name="s1x", bufs=2) as xpool,
                tc.tile_pool(name="s1t", bufs=4) as tpool,
            ):
                w_sbs = []
                for wi, (wten, wname) in enumerate(
                    ((wqT, "wq"), (wkT, "wk"), (wvT, "wv"))
                ):
                    wsb = wpool.tile([128, dt_, F_LOC], F32R, tag=wname)
                    src = wten[:, :].rearrange("(dt p) f -> p dt f", p=128).bitcast(F32R)
                    # wq on the sync ring ahead of x chunk 0; wk/wv on the
                    # scalar HWDGE ring so the first projections start early
                    if wi == 0:
                        nc.sync.dma_start(out=wsb, in_=src)
                    else:
                        nc.scalar.dma_start(out=wsb, in_=src)
                    w_sbs.append(wsb)

                for tch_i in range(ntch):
                    bi = tch_i // cpb
                    off = (tch_i % cpb) * tch
                    lsl = slice(off, off + tch)
                    tsl = slice(tch_i * tch, (tch_i + 1) * tch)
                    x_sb = xpool.tile([128, dt_, tch], F32R, tag="x")
                    xsrc = (
                        xT[:, tsl]
                        .rearrange("(dt p) tt -> p dt tt", p=128)
                        .bitcast(F32R)
                    )
                    if tch_i == 0:
                        # split the first chunk's load so the very first
                        # matmuls start after 1/4 of the transfer
                        step = dt_ // 4 if dt_ % 4 == 0 else dt_
                        for d0 in range(0, dt_, step):
                            nc.sync.dma_start(
                                out=x_sb[:, d0 : d0 + step, :],
                                in_=xsrc[:, d0 : d0 + step, :],
                            )
                    else:
                        nc.sync.dma_start(out=x_sb, in_=xsrc)
                    for pi in range(3):
                        for ft in range(HPC):
                            fsl = slice(ft * 128, (ft + 1) * 128)
                            ps = ps_mm.tile([128, tch], F32, tag="mm")
                            for di in range(dt_):
                                nc.tensor.matmul(
                                    ps,
                                    w_sbs[pi][:, di, fsl],
                                    x_sb[:, di, :],
                                    start=(di == 0),
                                    stop=(di == dt_ - 1),
                                )
                            if pi < 2:
                                # rope: out = in*cos + rot_half(in)*sin
                                ro = tpool.tile([128, tch], F32, tag="ro")
                                nc.vector.tensor_mul(
                                    ro, ps, cos_sb[:, lsl]
                                )
                                rt = tpool.tile([128, tch], F32, tag="rt")
                                nc.vector.tensor_mul(
                                    rt[0:64], ps[64:128], rsin_sb[0:64, lsl]
                                )
                                nc.vector.tensor_mul(
                                    rt[64:128], ps[0:64], rsin_sb[64:128, lsl]
                                )
                                nc.vector.tensor_add(ro, ro, rt)
                                scr = q_scr if pi == 0 else k_scr
                                nc.gpsimd.dma_start(
                                    out=scr[ft][bi][:, lsl], in_=ro
                                )
                            else:
                                vsb = tpool.tile([128, tch], F32, tag="vs")
                                nc.scalar.copy(vsb, ps)
                                for j in range(tch // 128):
                                    pst = ps_pv.tile([128, 128], F32, tag="pv")
                                    nc.tensor.transpose(
                                        pst, vsb[:, j * 128 : (j + 1) * 128], id_sb
                                    )
                                    vt = tpool.tile([128, 128], F32, tag="vt")
                                    nc.vector.tensor_copy(vt, pst)
                                    nc.gpsimd.dma_start(
                                        out=v_scr[ft][bi][
                                            (tch_i % cpb) * (tch // 128) + j, :, :
                                        ],
                                        in_=vt,
                                    )

            # ======== stage 2+3: attention + fused out-projection =========
            with (
                tc.tile_pool(name="s2in", bufs=2) as s2in,
                tc.tile_pool(name="s2", bufs=2) as s2pool,
                tc.tile_pool(name="s2e", bufs=8) as epool,
                tc.tile_pool(name="s3w", bufs=1) as wopool,
                tc.tile_pool(name="s3o", bufs=6) as s3pool,
            ):
                # out-projection psum rotation across every pool/tag: 8 banks
                # of recycling distance so evacuations never stall the PE
                s3_psrc = [
                    (ps_mm, "mm"), (ps_pv, "pv"), (ps_dn, "dn"), (ps_mm, "mm"),
                    (ps_pv, "pv"), (ps_dn, "dn"), (ps_mm, "mm"), (ps_mm, "mm"),
                ]
                # preload the out-projection weights so stage 3 starts hot
                wo_sb = wopool.tile([128, HPC, d_model], F32R, tag="wo")
                nc.sync.dma_start(
                    out=wo_sb,
                    in_=woT[:, :]
                    .rearrange("(ft p) d -> p ft d", p=128)
                    .bitcast(F32R),
                )
                for bi in range(b):
                    # normalized attention for this batch, f32r, feeds the
                    # fused out-projection directly from SBUF
                    attn_n = s2pool.tile([128, HPC, t], F32R, tag="an")
                    for h in range(HPC):
                        q_sb = s2in.tile([128, t], F32R, tag="q")
                        nc.sync.dma_start(
                            out=q_sb, in_=q_scr[h][bi][:, :].bitcast(F32R)
                        )
                        k_sb = s2in.tile([128, t], F32R, tag="k")
                        nc.sync.dma_start(
                            out=k_sb, in_=k_scr[h][bi][:, :].bitcast(F32R)
                        )
                        v_sb = s2in.tile([128, kt, 128], F32R, tag="v")
                        nc.sync.dma_start(
                            out=v_sb,
                            in_=v_scr[h][bi][:, :, :]
                            .rearrange("tt p dh -> p tt dh")
                            .bitcast(F32R),
                        )
                        attn_u = s2pool.tile([128, t], F32, tag="au")
                        den = s2pool.tile([1, t], F32, tag="den")
                        for qc in range(nqc):
                            qsl = slice(qc * qch, (qc + 1) * qch)
                            e_tiles = [None] * kt

                            def emit_score(kti):
                                sps = ps_mm.tile([128, qch], F32, tag="mm")
                                nc.tensor.matmul(
                                    sps,
                                    k_sb[:, kti * 128 : (kti + 1) * 128],
                                    q_sb[:, qsl],
                                    start=True,
                                    stop=True,
                                )
                                e_sb = epool.tile([128, qch], F32R, tag="E")
                                nc.scalar.activation(e_sb, sps, EXP, scale=SCALE)
                                e_tiles[kti] = e_sb

                            for kti in range(min(S_LOOK, kt)):
                                emit_score(kti)
                            pv = ps_pv.tile([128, qch], F32, tag="pv")
                            dn = ps_dn.tile([1, qch], F32, tag="dn")
                            for kti in range(kt):
                                nc.tensor.matmul(
                                    pv,
                                    v_sb[:, kti, :],
                                    e_tiles[kti],
                                    start=(kti == 0),
                                    stop=(kti == kt - 1),
                                )
                                nc.tensor.matmul(
                                    dn,
                                    ones_sb,
                                    e_tiles[kti],
                                    start=(kti == 0),
                                    stop=(kti == kt - 1),
                                )
                                if kti + S_LOOK < kt:
                                    emit_score(kti + S_LOOK)
                            nc.vector.tensor_copy(attn_u[:, qsl], pv)
                            nc.vector.tensor_copy(den[:, qsl], dn)
                        # normalize by softmax denominator. The reciprocal is
                        # computed on a [128, t/128] reshape of the row (a
                        # serial [1, t] reciprocal would clog the in-order DVE
                        # queue for ~13us).
                        drow = bi * HPC + h
                        nc.gpsimd.dma_start(
                            out=den_dram[drow : drow + 1, :], in_=den
                        )
                        rsm = s2pool.tile([128, t // 128], F32, tag="rsm")
                        nc.gpsimd.dma_start(
                            out=rsm,
                            in_=den_dram[drow, :].rearrange("(p i) -> p i", p=128),
                        )
                        nc.vector.reciprocal(rsm, rsm)
                        nc.gpsimd.dma_start(
                            out=rec_dram[drow, :].rearrange("(p i) -> p i", p=128),
                            in_=rsm,
                        )
                        rec = s2pool.tile([128, t], F32, tag="rec")
                        dsrc = rec_dram[drow : drow + 1, :]
                        bcast = bass.AP(
                            tensor=dsrc.tensor,
                            offset=dsrc.offset,
                            ap=[[0, 128]] + [list(p) for p in dsrc.ap[1:]],
                        )
                        nc.gpsimd.dma_start(out=rec, in_=bcast)
                        nc.vector.tensor_mul(attn_n[:, h, :], attn_u, rec)

                    # ---- fused out-projection for this batch ----
                    for c4 in range(cpb):
                        off = c4 * tch
                        gsl = slice(bi * t + off, bi * t + off + tch)
                        for do in range(dt_):
                            pool_, ptag = s3_psrc[do % 8]
                            ps = pool_.tile([128, tch], F32, tag=ptag)
                            for ft in range(HPC):
                                nc.tensor.matmul(
                                    ps,
                                    wo_sb[:, ft, do * 128 : (do + 1) * 128],
                                    attn_n[:, ft, off : off + tch],
                                    start=(ft == 0),
                                    stop=(ft == HPC - 1),
                                )
                            osb = s3pool.tile([128, tch], F32, tag="o")
                            if do % 2 == 0:
                                nc.vector.tensor_copy(osb, ps)
                                nc.gpsimd.dma_start(
                                    out=outP[do * 128 : (do + 1) * 128, gsl],
                                    in_=osb,
                                )
                            else:
                                nc.scalar.copy(osb, ps)
                                nc.sync.dma_start(
                                    out=outP[do * 128 : (do + 1) * 128, gsl],
                                    in_=osb,
                                )

    nc.finalize()
    return nc


_module_cache = {}


def _get_module(b, t, d_model, n_cores):
    key = (b, t, d_model, n_cores)
    if key not in _module_cache:
        _module_cache[key] = build_module(b, t, d_model, n_cores)
    return _module_cache[key]


def _host_tables(t):
    half = HEAD_DIM // 2
    theta = 1.0 / (
        np.float32(ROPE_BASE)
        ** (np.arange(half, dtype=np.float32) / np.float32(half))
    )
    freqs = np.arange(t, dtype=np.float32)[:, None] * theta[None, :]
    emb = np.concatenate([freqs, freqs], axis=-1)  # (t, 128)
    cosT = np.ascontiguousarray(np.cos(emb).T.astype(np.float32))
    sinT = np.sin(emb).T.astype(np.float32)
    rsinT = sinT.copy()
    rsinT[:half] = -sinT[:half]
    rsinT = np.ascontiguousarray(rsinT)
    return cosT, rsinT


def _run(x, Wq, Wk, Wv, Wo, trace=False):
    b_, t_, d_ = x.shape
    n_cores = (d_ // HEAD_DIM) // HPC
    nc = _get_module(b_, t_, d_, n_cores)

    xT = np.ascontiguousarray(x.reshape(b_ * t_, d_).T)
    cosT, rsinT = _host_tables(t_)
    ident = np.eye(128, dtype=np.float32)
    onesc = np.ones((128, 1), dtype=np.float32)

    in_maps = []
    for c in range(n_cores):
        fs = slice(c * F_LOC, (c + 1) * F_LOC)
        in_maps.append(
            {
                "xT": xT,
                "wqT": np.ascontiguousarray(Wq[fs, :].T),
                "wkT": np.ascontiguousarray(Wk[fs, :].T),
                "wvT": np.ascontiguousarray(Wv[fs, :].T),
                "woT": np.ascontiguousarray(Wo[:, fs].T),
                "cosT": cosT,
                "rsinT": rsinT,
                "ident": ident,
                "onesc": onesc,
            }
        )
    res = run_bass_kernel_spmd(
        nc, in_maps, core_ids=list(range(n_cores)), trace=trace
    )
    acc = res.results[0]["outP"].copy()
    for c in range(1, n_cores):
        acc += res.results[c]["outP"]
    out = np.ascontiguousarray(acc.T).reshape(b_, t_, d_)
    return out, res


def kernel(x, Wq, Wk, Wv, Wo):
    x = np.asarray(x, dtype=np.float32)
    Wq = np.asarray(Wq, dtype=np.float32)
    Wk = np.asarray(Wk, dtype=np.float32)
    Wv = np.asarray(Wv, dtype=np.float32)
    Wo = np.asarray(Wo, dtype=np.float32)
    out, _ = _run(x, Wq, Wk, Wv, Wo, trace=False)
    return out



# revision 17
# speedup vs baseline: 1.1340x; 1.1340x over previous
"""Trainium2 Bass kernel: 16-head RoPE attention block (B=4, T=2048, D=2048).

Sharding: tensor-parallel over heads. Each of the 8 cores owns 2 heads
(a 256-wide slice of the q/k/v projection output features). Host sums
the 8 partial out-projection products (the "all-reduce").

v2 design (vs the two-pass baseline):
  - All matmul operands in bf16 (fp32 PSUM accumulate): halves DMA,
    enables fast weight load. PSUM/softmax math stays fp32.
  - Per-batch fusion: q/k/v live in SBUF (no DRAM scratch round-trip).
  - Interleaved emission s1(b0) s2(b0) s1(b1) s3(b0) s2(b1) ... keeps
    the PE queue full across batch boundaries (no HAM re-throttle).
  - Softmax normalization stays on-chip: dn psum rows are re-partitioned
    to [128,16] via a tiny PSUM->SBUF DMA, reciprocal on the vector
    engine, scattered back to a [1,T] row, and broadcast to [128,T] with
    a rank-1 ones matmul on the PE (no DRAM, no 13us serial reciprocal).
  - RoPE rotate-half multiplies run on the (otherwise idle) gpsimd
    engine; cos-mul + add on the vector engine.
"""

import math

import numpy as np
import ml_dtypes

import concourse.bacc as bacc
import concourse.bass as bass
import concourse.mybir as mybir
import concourse.tile as tile
from concourse.bass_utils import run_bass_kernel_spmd

F32 = mybir.dt.float32
F32R = mybir.dt.float32r
BF16 = mybir.dt.bfloat16
EXP = mybir.ActivationFunctionType.Exp
COPY = mybir.ActivationFunctionType.Copy

# Problem shape (hardcoded; the harness calls kernel() with exactly these).
B = 4
T = 2048
D_MODEL = 2048
HEAD_DIM = 128
N_CORES = 8
ROPE_BASE = 10000.0

HPC = 2                      # heads per core
F_LOC = HPC * HEAD_DIM       # 256 local projection features per core
TCH = 512                    # token chunk width (stages 1/3)
QCH = 512                    # query chunk width (stage 2)
SCALE = 1.0 / math.sqrt(HEAD_DIM)
S_LOOK = 3                   # score-matmul lookahead in the attention loop

NPBF16 = ml_dtypes.bfloat16


def build_module(b=B, t=T, d_model=D_MODEL):
    bt = b * t
    dt_ = d_model // 128     # 16 contraction tiles
    kt = t // 128            # 16 key tiles per (batch, head)
    tch = TCH
    qch = QCH
    nqc = t // qch           # 4 query chunks
    cpb = t // tch           # 4 stage-1/3 token chunks per batch

    nc = bacc.Bacc(None, target_bir_lowering=False)

    xT = nc.dram_tensor("xT", [d_model, bt], BF16, kind="ExternalInput")
    wqT = nc.dram_tensor("wqT", [d_model, F_LOC], BF16, kind="ExternalInput")
    wkT = nc.dram_tensor("wkT", [d_model, F_LOC], BF16, kind="ExternalInput")
    wvT = nc.dram_tensor("wvT", [d_model, F_LOC], BF16, kind="ExternalInput")
    woT = nc.dram_tensor("woT", [F_LOC, d_model], BF16, kind="ExternalInput")
    cosT = nc.dram_tensor("cosT", [HEAD_DIM, t], F32, kind="ExternalInput")
    rsinT = nc.dram_tensor("rsinT", [HEAD_DIM, t], F32, kind="ExternalInput")
    ident = nc.dram_tensor("ident", [128, 128], F32, kind="ExternalInput")
    onesd = nc.dram_tensor("onesd", [128, 1], BF16, kind="ExternalInput")
    onesb = nc.dram_tensor("onesb", [1, 128], F32, kind="ExternalInput")
    outP = nc.dram_tensor("outP", [d_model, bt], BF16, kind="ExternalOutput")

    with tile.TileContext(nc) as tc:
        with (
            tc.tile_pool(name="const", bufs=1) as constp,
            tc.tile_pool(name="wpool", bufs=1) as wpool,
            tc.tile_pool(name="xpool", bufs=2) as xpool,
            tc.tile_pool(name="qkv", bufs=2) as qkvp,
            tc.tile_pool(name="rope", bufs=2) as ropep,
            tc.tile_pool(name="vstg", bufs=2) as vstg,
            tc.tile_pool(name="epool", bufs=6) as epool,
            tc.tile_pool(name="attn", bufs=2) as attnp,
            tc.tile_pool(name="nrm", bufs=1) as nrmp,
            tc.tile_pool(name="opool", bufs=6) as opool,
            tc.tile_pool(name="ps_mm", bufs=3, space="PSUM") as ps_mm,
            tc.tile_pool(name="ps_pv", bufs=2, space="PSUM") as ps_pv,
            tc.tile_pool(name="ps_dn", bufs=2, space="PSUM") as ps_dn,
            tc.tile_pool(name="ps_bc", bufs=1, space="PSUM") as ps_bc,
        ):
            # ---- constants + weights ----
            # sync ring: wq + first x chunk interleaved (gates first matmuls)
            # scalar ring: wk, wv (needed a few us in), wo later
            # gpsimd ring: cos/rsin (rope, ~4us in), identity/ones
            wq_sb = wpool.tile([128, dt_, F_LOC], BF16, tag="wq")
            wk_sb = wpool.tile([128, dt_, F_LOC], BF16, tag="wk")
            wv_sb = wpool.tile([128, dt_, F_LOC], BF16, tag="wv")
            x_sb0 = xpool.tile([128, dt_, tch], BF16, tag="x")
            xsrc0 = xT[:, 0:tch].rearrange("(dt p) tt -> p dt tt", p=128)
            qtr = dt_ // 4
            for q4 in range(4):
                dsl = slice(q4 * qtr, (q4 + 1) * qtr)
                nc.sync.dma_start(
                    out=wq_sb[:, dsl, :],
                    in_=wqT[:, :].rearrange("(dt p) f -> p dt f", p=128)[:, dsl, :],
                )
                nc.sync.dma_start(out=x_sb0[:, dsl, :], in_=xsrc0[:, dsl, :])
            nc.scalar.dma_start(
                out=wk_sb, in_=wkT[:, :].rearrange("(dt p) f -> p dt f", p=128)
            )
            nc.scalar.dma_start(
                out=wv_sb, in_=wvT[:, :].rearrange("(dt p) f -> p dt f", p=128)
            )
            cos_sb = constp.tile([128, t], F32)
            nc.gpsimd.dma_start(out=cos_sb, in_=cosT[:, :])
            rsin_sb = constp.tile([128, t], F32)
            nc.gpsimd.dma_start(out=rsin_sb, in_=rsinT[:, :])
            id_sb = constp.tile([128, 128], F32)
            nc.gpsimd.dma_start(out=id_sb, in_=ident[:, :])
            onesd_sb = constp.tile([128, 1], BF16)
            nc.gpsimd.dma_start(out=onesd_sb, in_=onesd[:, :])
            onesb_sb = constp.tile([1, 128], F32R)
            nc.gpsimd.dma_start(out=onesb_sb, in_=onesb[:, :].bitcast(F32R))
            wo_sb = wpool.tile([128, HPC, d_model], BF16, tag="wo")
            nc.scalar.dma_start(
                out=wo_sb, in_=woT[:, :].rearrange("(ft p) d -> p ft d", p=128)
            )
            w_sbs = [wq_sb, wk_sb, wv_sb]

            # stage-3 psum rotation across pools (recycling distance)
            s3_psrc = [
                (ps_mm, "mm"), (ps_pv, "pv"), (ps_dn, "dn"), (ps_mm, "mm"),
                (ps_pv, "pv"), (ps_dn, "dn"), (ps_mm, "mm"), (ps_bc, "bc"),
            ]

            def emit_s1(bi):
                """Projections + rope + v-transpose for batch bi.
                Returns (q_sb, k_sb, v_sb) bf16 SBUF tiles."""
                q_sb = qkvp.tile([128, HPC, t], BF16, tag="q")
                k_sb = qkvp.tile([128, HPC, t], BF16, tag="k")
                v_sb = qkvp.tile([128, HPC, kt, 128], BF16, tag="v")
                qk_dst = [q_sb, k_sb]
                for ci in range(cpb):
                    off = ci * tch
                    lsl = slice(off, off + tch)
                    tsl = slice(bi * t + off, bi * t + off + tch)
                    if bi == 0 and ci == 0:
                        x_sb = x_sb0
                    else:
                        x_sb = xpool.tile([128, dt_, tch], BF16, tag="x")
                        nc.sync.dma_start(
                            out=x_sb,
                            in_=xT[:, tsl].rearrange(
                                "(dt p) tt -> p dt tt", p=128
                            ),
                        )
                    for pi in range(3):
                        for ft in range(HPC):
                            fsl = slice(ft * 128, (ft + 1) * 128)
                            ps = ps_mm.tile([128, tch], F32, tag="mm")
                            for di in range(dt_):
                                nc.tensor.matmul(
                                    ps,
                                    w_sbs[pi][:, di, fsl],
                                    x_sb[:, di, :],
                                    start=(di == 0),
                                    stop=(di == dt_ - 1),
                                )
                            if pi < 2:
                                # rope: out = in*cos + rot_half(in)*rsin
                                # rt muls on gpsimd (idle), rest on DVE
                                ro = ropep.tile([128, tch], F32, tag="ro")
                                nc.vector.tensor_mul(ro, ps, cos_sb[:, lsl])
                                rt = ropep.tile([128, tch], F32, tag="rt")
                                nc.vector.tensor_mul(
                                    rt[0:64], ps[64:128], rsin_sb[0:64, lsl]
                                )
                                nc.vector.tensor_mul(
                                    rt[64:128], ps[0:64], rsin_sb[64:128, lsl]
                                )
                                nc.vector.tensor_add(
                                    qk_dst[pi][:, ft, lsl], ro, rt
                                )
                            else:
                                vsb = vstg.tile([128, tch], F32, tag="vs")
                                nc.scalar.activation(vsb, ps, COPY)
                                pst = ps_pv.tile([128, tch], F32, tag="pv")
                                for j in range(tch // 128):
                                    nc.tensor.transpose(
                                        pst[:, j * 128 : (j + 1) * 128],
                                        vsb[:, j * 128 : (j + 1) * 128],
                                        id_sb,
                                    )
                                nc.vector.tensor_copy(
                                    v_sb[:, ft, ci * 4 : (ci + 1) * 4, :]
                                    .rearrange("p a b -> p (a b)"),
                                    pst,
                                )
                return q_sb, k_sb, v_sb

            def emit_s2(bi, h, qkv, attn_n):
                """Attention for (batch bi, local head h) -> writes
                attn_n[:, h, :] (normalized, bf16)."""
                q_sb, k_sb, v_sb = qkv
                attn_u = attnp.tile([128, t], F32, tag="au")
                den_row = nrmp.tile([1, t], F32, tag="dr")
                den_t = nrmp.tile([128, t // 128], F32, tag="dT")
                rec_t = nrmp.tile([128, t // 128], F32, tag="rT")
                rec_row = nrmp.tile([1, t], F32R, tag="rr")
                dn_tiles = []
                for qc in range(nqc):
                    qsl = slice(qc * qch, (qc + 1) * qch)
                    e_tiles = [None] * kt

                    def emit_score(kti):
                        sps = ps_mm.tile([128, qch], F32, tag="mm")
                        nc.tensor.matmul(
                            sps,
                            k_sb[:, h, kti * 128 : (kti + 1) * 128],
                            q_sb[:, h, qsl],
                            start=True,
                            stop=True,
                        )
                        e_sb = epool.tile([128, qch], BF16, tag="E")
                        nc.scalar.activation(e_sb, sps, EXP, scale=SCALE)
                        e_tiles[kti] = e_sb

                    for kti in range(min(S_LOOK, kt)):
                        emit_score(kti)
                    pv = ps_pv.tile([128, qch], F32, tag="pv")
                    dn = ps_dn.tile([128, qch], F32, tag="dn")
                    for kti in range(kt):
                        nc.tensor.matmul(
                            pv,
                            v_sb[:, h, kti, :],
                            e_tiles[kti],
                            start=(kti == 0),
                            stop=(kti == kt - 1),
                        )
                        nc.tensor.matmul(
                            dn[0:1, :],
                            onesd_sb,
                            e_tiles[kti],
                            start=(kti == 0),
                            stop=(kti == kt - 1),
                        )
                        if kti + S_LOOK < kt:
                            emit_score(kti + S_LOOK)
                    nc.vector.tensor_copy(attn_u[:, qsl], pv)
                    nc.vector.tensor_copy(den_row[:, qsl], dn[0:1, :])
                    dn_tiles.append(dn)
                # re-partition the [1,t] denominator row to [128, t/128]
                # (element m of the row -> (m // 16, m % 16)), reciprocal in
                # the parallel layout, scatter back with the inverse mapping.
                nc.gpsimd.dma_start(out=den_t, in_=den_row)
                nc.vector.reciprocal(rec_t, den_t)
                nc.gpsimd.dma_start(
                    out=rec_row, in_=rec_t[:, :].bitcast(F32R)
                )
                for qc in range(nqc):
                    qsl = slice(qc * qch, (qc + 1) * qch)
                    rbc = ps_bc.tile([128, qch], F32, tag="bc")
                    nc.tensor.matmul(
                        rbc,
                        onesb_sb,
                        rec_row[:, qsl],
                        start=True,
                        stop=True,
                    )
                    nc.vector.tensor_mul(attn_n[:, h, qsl], attn_u[:, qsl], rbc)

            def emit_s3(bi, attn_n):
                """Fused out-projection for batch bi (partial product over
                this core's 256 features, full d_model columns)."""
                for c4 in range(cpb):
                    off = c4 * tch
                    gsl = slice(bi * t + off, bi * t + off + tch)
                    for do in range(dt_):
                        pool_, ptag = s3_psrc[do % 8]
                        ps = pool_.tile([128, tch], F32, tag=ptag)
                        for ft in range(HPC):
                            nc.tensor.matmul(
                                ps,
                                wo_sb[:, ft, do * 128 : (do + 1) * 128],
                                attn_n[:, ft, off : off + tch],
                                start=(ft == 0),
                                stop=(ft == HPC - 1),
                            )
                        osb = opool.tile([128, tch], BF16, tag="o")
                        if do % 2 == 0:
                            nc.vector.tensor_copy(osb, ps)
                            nc.gpsimd.dma_start(
                                out=outP[do * 128 : (do + 1) * 128, gsl],
                                in_=osb,
                            )
                        else:
                            nc.scalar.activation(osb, ps, COPY)
                            nc.sync.dma_start(
                                out=outP[do * 128 : (do + 1) * 128, gsl],
                                in_=osb,
                            )

            # ---- interleaved schedule: PE never drains at batch edges ----
            qkv = emit_s1(0)
            attn_prev = None
            for bi in range(b):
                attn_n = attnp.tile([128, HPC, t], BF16, tag="an")
                emit_s2(bi, 0, qkv, attn_n)
                emit_s2(bi, 1, qkv, attn_n)
                if bi + 1 < b:
                    qkv = emit_s1(bi + 1)
                emit_s3(bi, attn_n)

    nc.finalize()
    return nc


_module_cache = {}


def _get_module(b, t, d_model):
    key = (b, t, d_model)
    if key not in _module_cache:
        _module_cache[key] = build_module(b, t, d_model)
    return _module_cache[key]


def _host_tables(t):
    half = HEAD_DIM // 2
    theta = 1.0 / (
        np.float32(ROPE_BASE)
        ** (np.arange(half, dtype=np.float32) / np.float32(half))
    )
    freqs = np.arange(t, dtype=np.float32)[:, None] * theta[None, :]
    emb = np.concatenate([freqs, freqs], axis=-1)  # (t, 128)
    cosT = np.ascontiguousarray(np.cos(emb).T.astype(np.float32))
    sinT = np.sin(emb).T.astype(np.float32)
    rsinT = sinT.copy()
    rsinT[:half] = -sinT[:half]
    rsinT = np.ascontiguousarray(rsinT)
    return cosT, rsinT


def _run(x, Wq, Wk, Wv, Wo, trace=False):
    b_, t_, d_ = x.shape
    n_cores = (d_ // HEAD_DIM) // HPC
    nc = _get_module(b_, t_, d_)

    xT = np.ascontiguousarray(
        x.reshape(b_ * t_, d_).T.astype(NPBF16)
    )
    cosT, rsinT = _host_tables(t_)
    ident = np.eye(128, dtype=np.float32)
    onesd = np.ones((128, 1), dtype=NPBF16)
    onesb = np.ones((1, 128), dtype=np.float32)

    in_maps = []
    for c in range(n_cores):
        fs = slice(c * F_LOC, (c + 1) * F_LOC)
        in_maps.append(
            {
                "xT": xT,
                "wqT": np.ascontiguousarray(Wq[fs, :].T.astype(NPBF16)),
                "wkT": np.ascontiguousarray(Wk[fs, :].T.astype(NPBF16)),
                "wvT": np.ascontiguousarray(Wv[fs, :].T.astype(NPBF16)),
                "woT": np.ascontiguousarray(Wo[:, fs].T.astype(NPBF16)),
                "cosT": cosT,
                "rsinT": rsinT,
                "ident": ident,
                "onesd": onesd,
                "onesb": onesb,
            }
        )
    res = run_bass_kernel_spmd(
        nc, in_maps, core_ids=list(range(n_cores)), trace=trace
    )
    acc = res.results[0]["outP"].astype(np.float32)
    for c in range(1, n_cores):
        acc += res.results[c]["outP"].astype(np.float32)
    out = np.ascontiguousarray(acc.T).reshape(b_, t_, d_)
    return out, res


def kernel(x, Wq, Wk, Wv, Wo):
    x = np.asarray(x, dtype=np.float32)
    Wq = np.asarray(Wq, dtype=np.float32)
    Wk = np.asarray(Wk, dtype=np.float32)
    Wv = np.asarray(Wv, dtype=np.float32)
    Wo = np.asarray(Wo, dtype=np.float32)
    out, _ = _run(x, Wq, Wk, Wv, Wo, trace=False)
    return out
